# revision 1
# baseline (speedup 1.0000x reference)
"""Trainium2 Bass kernel for nn_DecoderTP_accu (Hawkes decoder losses).

Strategy (8 NeuronCores, data-parallel):
  - The dominant work is the survival-sample Hawkes intensity over
    u_non/v_non (131072, 256) f32 each (268 MB streamed once) -> shard the
    S*N rows 16384/core (each core gets 2 full survival samples s=2c,2c+1).
  - The event path (8192 events) is sharded 1024 events/core; z_src/z_dst
    rows are gathered on host (tiny) and streamed on device.
  - Per row the device computes softplus(clip((u.Wu + v.Wv + b +
    alpha*exp(-w_t*td/5000)) / (psi+1e-7), -75, 75)) using one fused
    multiply+reduce (tensor_tensor_reduce) per 128-row group per operand.
  - Host does the 8192-sized index gathers (assoc/src/pos_dst,
    event_inten_accu lookup), the mean over s, and the two scalar losses.

Row mapping per core (surv path): flat row i (= s_local*8192 + n) lives at
big-tile bt = i // 2048, partition p = (i % 2048) // 16, group j = i % 16.
Event mapping per core: event k lives at partition k // 8, group k % 8.
"""

import numpy as np

E = 256
S = 16
N = 8192
NCORES = 8
ROWS = S * N // NCORES      # 16384 rows/core
BT = 8                      # big tiles per core (surv path)
J = 16                      # 128-row groups per big tile
EV = N // NCORES            # 1024 events/core
JE = 8                      # groups in the event tile
TD_HR_MAX = 5000.0
MIN_DST = 10000

_CACHE = {}


def _build_module(btn=BT, jn=J, evn=EV, jen=JE):
    key = (btn, jn, evn, jen)
    if key in _CACHE:
        return _CACHE[key]

    import concourse.bacc as bacc
    import concourse.tile as tile
    from concourse import mybir
    from concourse.hw_specs import get_activation_tables

    f32 = mybir.dt.float32
    A = mybir.AluOpType
    F = mybir.ActivationFunctionType

    class _Bacc(bacc.Bacc):
        # The stock table chooser takes the first act-table set containing
        # each function; Exp and Ln land in different sets and the ACT
        # engine thrashes ~1.3us table reloads. Put the set holding both
        # first so every activation here resolves to one table.
        def insert_act_table_loads(self):
            has_activation = any(
                isinstance(i, mybir.InstActivation)
                for b in self.main_func.blocks
                for i in b.instructions
            )
            if not has_activation:
                return
            tables = get_activation_tables(self.m.arch)
            # keep positional ids intact; just hide Ln from the earlier
            # 'natural_log' set so first-match picks the Exp+Ln set for both
            order = [
                (name, funcs - {mybir.ActivationFunctionType.Ln}
                 if name == "natural_log" else funcs)
                for name, funcs in tables.items()
            ]
            import bass_rust as _bass_rust

            _bass_rust.insert_act_table_loads(self, order)

    nc = _Bacc(None, target_bir_lowering=False)

    bf16 = mybir.dt.bfloat16
    rows_n = 128 * btn * jn
    uv_d = nc.dram_tensor("uv", [rows_n, 2 * E], bf16, kind="ExternalInput")
    z_d = nc.dram_tensor("z", [evn, 2 * E], f32, kind="ExternalInput")
    td_d = nc.dram_tensor("td", [128, btn * jn], f32, kind="ExternalInput")
    tduv_d = nc.dram_tensor("tduv", [128, jen], f32, kind="ExternalInput")
    w_d = nc.dram_tensor("wvec", [1, 2 * E], f32, kind="ExternalInput")
    sc_d = nc.dram_tensor("scal", [1, 4], f32, kind="ExternalInput")

    osurv_d = nc.dram_tensor("osurv", [128, btn * jn], f32, kind="ExternalOutput")
    oev_d = nc.dram_tensor("oev", [128, jen], f32, kind="ExternalOutput")

    with tile.TileContext(nc) as tc:
        with (
            tc.tile_pool(name="const", bufs=1) as cp,
            tc.tile_pool(name="uin", bufs=3) as up,
            tc.tile_pool(name="vin", bufs=3) as vp,
            tc.tile_pool(name="scr", bufs=2) as scr,
            tc.tile_pool(name="small", bufs=4) as sm,
            tc.tile_pool(name="outs", bufs=1) as op,
        ):
            wb32 = cp.tile([128, 2 * E], f32)
            nc.gpsimd.dma_start(out=wb32[:], in_=w_d[:].to_broadcast([128, 2 * E]))
            wb16 = cp.tile([128, 2 * E], bf16)
            nc.vector.tensor_copy(out=wb16[:], in_=wb32[:])
            sc = cp.tile([128, 4], f32)
            nc.gpsimd.dma_start(out=sc[:], in_=sc_d[:].to_broadcast([128, 4]))

            # per-partition scalars: b, alpha, -w_t/TD_HR_MAX, 1/(psi+1e-7)
            al_col = sc[:, 1:2]
            esc = cp.tile([128, 1], f32)
            nc.vector.tensor_scalar_mul(out=esc[:], in0=sc[:, 2:3],
                                        scalar1=-1.0 / TD_HR_MAX)
            pse = cp.tile([128, 1], f32)
            nc.vector.tensor_scalar_add(out=pse[:], in0=sc[:, 3:4], scalar1=1e-7)
            ivp = cp.tile([128, 1], f32)
            nc.vector.reciprocal(out=ivp[:], in_=pse[:])
            bivp = cp.tile([128, 1], f32)
            nc.vector.tensor_mul(out=bivp[:], in0=sc[:, 0:1], in1=ivp[:])

            tdt = cp.tile([128, btn * jn], f32)
            nc.sync.dma_start(out=tdt[:], in_=td_d[:])
            tut = cp.tile([128, jen], f32)
            nc.sync.dma_start(out=tut[:], in_=tduv_d[:])

            osurv_t = op.tile([128, btn * jn], f32)
            oev_t = op.tile([128, jen], f32)

            A_EVERY = 3  # 1 of every A_EVERY groups reduces on DVE; rest on ACT

            def hawkes_tile(uvt, nj, td_ap, out_ap, wb, split):
                # uvt: [128, nj, 2E] sbuf tile with u rows in [:, :, 0:E] and
                # v rows in [:, :, E:2E]; td_ap: [128, nj] time deltas;
                # out_ap: [128, nj] destination for softplus(g_psi).
                # gs[:, j] = sum(u*Wu) + sum(v*Wv), computed either fully on
                # DVE (stt + accumulator) or as DVE 2x bf16 mult + ACT
                # copy-accumulate, to balance the two engines.
                gs = sm.tile([128, nj], f32, tag="gs")
                for j in range(nj):
                    if not split or j % A_EVERY == 0:
                        s1 = scr.tile([128, 2 * E], uvt.dtype, tag="s1")
                        nc.vector.scalar_tensor_tensor(
                            out=s1[:], in0=uvt[:, j, :], scalar=1.0, in1=wb[:],
                            op0=A.mult, op1=A.mult, accum_out=gs[:, j : j + 1],
                        )
                    else:
                        s2 = scr.tile([128, 2 * E], uvt.dtype, tag="s2")
                        nc.vector.tensor_mul(out=s2[:], in0=uvt[:, j, :],
                                             in1=wb[:])
                        nc.scalar.activation(out=s2[:], in_=s2[:], func=F.Copy,
                                             accum_out=gs[:, j : j + 1])
                et = sm.tile([128, nj], f32, tag="et")
                nc.scalar.activation(out=et[:], in_=td_ap, func=F.Exp,
                                     scale=esc[:, 0:1], bias=0.0)
                g2 = sm.tile([128, nj], f32, tag="g2")
                nc.vector.scalar_tensor_tensor(
                    out=g2[:], in0=et[:], scalar=al_col, in1=gs[:],
                    op0=A.mult, op1=A.add,
                )
                # c1 = (g2 + b) / (psi + 1e-7) = g2*ivp + b*ivp
                c1 = sm.tile([128, nj], f32, tag="c1")
                nc.vector.tensor_scalar(
                    out=c1[:], in0=g2[:], scalar1=ivp[:, 0:1],
                    scalar2=bivp[:, 0:1], op0=A.mult, op1=A.add,
                )
                c1b = sm.tile([128, nj], f32, tag="c1b")
                nc.vector.tensor_scalar_min(out=c1b[:], in0=c1[:], scalar1=75.0)
                c2 = sm.tile([128, nj], f32, tag="c2")
                nc.vector.tensor_scalar_max(out=c2[:], in0=c1b[:], scalar1=-75.0)
                # softplus(x) = relu(x) + ln(1 + exp(min(x, -x)))
                # (no Softplus table on gen3; ACT runs only Exp/Ln -> 1 table)
                nx = sm.tile([128, nj], f32, tag="nx")
                nc.vector.tensor_scalar_mul(out=nx[:], in0=c2[:], scalar1=-1.0)
                mn = sm.tile([128, nj], f32, tag="mn")
                nc.vector.tensor_tensor(out=mn[:], in0=c2[:], in1=nx[:],
                                        op=A.min)
                e3 = sm.tile([128, nj], f32, tag="e3")
                nc.scalar.activation(out=e3[:], in_=mn[:], func=F.Exp)
                l1 = sm.tile([128, nj], f32, tag="l1")
                nc.scalar.activation(out=l1[:], in_=e3[:], func=F.Ln, bias=1.0)
                rl = sm.tile([128, nj], f32, tag="rl")
                nc.vector.tensor_scalar_max(out=rl[:], in0=c2[:], scalar1=0.0)
                nc.vector.tensor_add(out=out_ap, in0=rl[:], in1=l1[:])

            for bt in range(btn):
                uvt = up.tile([128, jn, 2 * E], bf16, tag="uvt")
                nc.sync.dma_start(
                    out=uvt[:],
                    in_=uv_d[bt * 128 * jn : (bt + 1) * 128 * jn, :].rearrange(
                        "(p j) f -> p j f", p=128
                    ),
                )
                hawkes_tile(uvt, jn, tdt[:, bt * jn : (bt + 1) * jn],
                            osurv_t[:, bt * jn : (bt + 1) * jn], wb16, True)

            zt = up.tile([128, jen, 2 * E], f32, tag="zt")
            nc.sync.dma_start(out=zt[:],
                              in_=z_d[:].rearrange("(p j) f -> p j f", p=128))
            hawkes_tile(zt, jen, tut[:], oev_t[:], wb32, False)

            nc.sync.dma_start(out=osurv_d[:], in_=osurv_t[:])
            nc.sync.dma_start(out=oev_d[:], in_=oev_t[:])

    nc.finalize()
    _CACHE[key] = nc
    return nc


def _stage_inputs(inputs):
    """Host-side prep: index gathers + per-core sharding. Returns
    (in_maps, td_uv, use_accu, accu_g, psi)."""
    all_embeddings = np.asarray(inputs["all_embeddings"], dtype=np.float32)
    assoc = np.asarray(inputs["assoc"])
    src = np.asarray(inputs["src"])
    pos_dst = np.asarray(inputs["pos_dst"])
    last_update = np.asarray(inputs["last_update"], dtype=np.float32)
    cur_time = np.asarray(inputs["cur_time"], dtype=np.float32)
    u_non = np.asarray(inputs["u_non_embeddings"], dtype=np.float32)
    v_non = np.asarray(inputs["v_non_embeddings"], dtype=np.float32)
    last_time_pos = np.asarray(inputs["last_time_pos"], dtype=np.float32)
    td_surv_step = np.asarray(inputs["td_surv_step"], dtype=np.float32)
    event_inten_accu = np.asarray(inputs["event_inten_accu"], dtype=np.float32)
    W_omega = np.asarray(inputs["W_omega"], dtype=np.float32)
    b_omega = np.asarray(inputs["b_omega"], dtype=np.float32)
    psi = np.asarray(inputs["psi"], dtype=np.float32)
    alpha = np.asarray(inputs["alpha"], dtype=np.float32)
    w_t = np.asarray(inputs["w_t"], dtype=np.float32)

    idx_src = assoc[src]
    idx_dst = assoc[pos_dst]
    lu_src = last_update[idx_src]
    lu_dst = last_update[idx_dst]
    lum = np.maximum(lu_src, lu_dst)
    use_accu = (last_time_pos >= lum).astype(np.float32)
    t_uv = np.maximum(lum, last_time_pos)
    td_uv = (cur_time - t_uv).astype(np.float32)

    td_non = (td_surv_step * td_uv[None, :]).astype(np.float32)  # (S, N)
    accu_g = event_inten_accu[src, pos_dst - MIN_DST].astype(np.float32)

    # interleave u|v per row so each device tile loads with one contiguous
    # DMA; bf16 halves both DMA bytes and DVE cycles (f32 accumulate)
    import ml_dtypes

    bf = ml_dtypes.bfloat16
    uv = np.empty((S * N, 2 * E), dtype=bf)
    uv[:, :E] = u_non
    uv[:, E:] = v_non
    z = np.empty((N, 2 * E), dtype=np.float32)
    z[:, :E] = all_embeddings[idx_src]
    z[:, E:] = all_embeddings[idx_dst]

    wvec = np.ascontiguousarray(W_omega.reshape(1, 2 * E))
    scal = np.array([[b_omega[0], alpha[0], w_t[0], psi[0]]], dtype=np.float32)

    in_maps = []
    for c in range(NCORES):
        td_core = td_non[2 * c : 2 * c + 2, :].reshape(-1)  # (16384,)
        td_staged = np.ascontiguousarray(
            td_core.reshape(BT, 128, J).transpose(1, 0, 2).reshape(128, BT * J)
        )
        tduv_staged = np.ascontiguousarray(
            td_uv[c * EV : (c + 1) * EV].reshape(128, JE)
        )
        in_maps.append(
            dict(
                uv=uv[c * ROWS : (c + 1) * ROWS],
                z=z[c * EV : (c + 1) * EV],
                td=td_staged,
                tduv=tduv_staged,
                wvec=wvec,
                scal=scal,
            )
        )
    return in_maps, td_uv, use_accu, accu_g, float(psi[0])


def _combine(results, td_uv, use_accu, accu_g, psi_val):
    sp_sum = np.zeros(N, dtype=np.float64)
    lam_ev = np.empty(N, dtype=np.float64)
    for c, r in enumerate(results):
        o = np.asarray(r["osurv"], dtype=np.float64)  # (128, BT*J)
        rows = o.reshape(128, BT, J).transpose(1, 0, 2).reshape(ROWS)
        sp_sum += rows.reshape(2, N).sum(axis=0)
        lam_ev[c * EV : (c + 1) * EV] = np.asarray(
            r["oev"], dtype=np.float64
        ).reshape(EV)

    mean_lambda_surv = psi_val * (sp_sum / S)
    integral = mean_lambda_surv * td_uv.astype(np.float64) + use_accu.astype(
        np.float64
    ) * accu_g.astype(np.float64)
    loss_surv = integral.sum() / N

    lam_uv = psi_val * lam_ev
    loss_lambda = -np.log(lam_uv + 1e-7).sum() / N
    return np.float32(loss_lambda), np.float32(loss_surv)


def _run(in_maps, trace=False):
    from concourse.bass_utils import run_bass_kernel_spmd

    nc = _build_module()
    res = run_bass_kernel_spmd(
        nc, in_maps, core_ids=list(range(NCORES)), trace=trace
    )
    return res


def kernel(**inputs):
    in_maps, td_uv, use_accu, accu_g, psi_val = _stage_inputs(inputs)
    res = _run(in_maps)
    return _combine(res.results, td_uv, use_accu, accu_g, psi_val)


def kernel_traced(**inputs):
    """Like kernel() but also returns the HW exec time in ns (test harness)."""
    in_maps, td_uv, use_accu, accu_g, psi_val = _stage_inputs(inputs)
    res = _run(in_maps, trace=True)
    out = _combine(res.results, td_uv, use_accu, accu_g, psi_val)
    return out, res.exec_time_ns



# revision 4
# speedup vs baseline: 1.9936x; 1.9936x over previous
"""Trainium2 Bass kernel for nn_DecoderTP_accu (Hawkes decoder losses).

Strategy (8 NeuronCores, data-parallel):
  - Dominant work: per-row dot products over u_non/v_non (131072 rows x 512
    dims). Shard rows 16384/core (2 survival samples per core) and run the
    dots on the otherwise-idle Tensor engine: host pre-arranges the data as
    fp8 PE-stationary tiles [128 dims, 128 rows]; w is the tiny moving
    operand ([128, 2, 1] per 256-dim half with DoubleRow packing), so each
    matmul emits one [128 rows, 1] PSUM column. 2 matmuls (256-dim halves)
    per 128-row group, 128 groups -> PSUM [128, 128] (partition = row in
    group, column = group).
  - fp8 halves DMA vs bf16 (8.4 MB/core); the error washes out in the
    131072-row mean.
  - Event path (8192 events, z_src|z_dst gathered on host): same shape in
    bf16, 4 chunk matmuls per 128-event group -> PSUM [128, 8].
  - DVE/ACT only run the pointwise tail on [128, 128]/[128, 8] tiles:
    g2 = alpha*exp(-w_t*td/5000) + g, clip, softplus via Ln(1+Exp(x))
    (clip to +-75*psi' first so Exp stays in range; b_omega folds into the
    activation bias).
  - Host does the index gathers, event_inten_accu lookup, *psi scaling,
    mean over s and the two scalar reductions (tiny).

Row mapping per core: flat row r (= s_local*8192 + n) sits at PSUM
partition r % 128, column r // 128. Events: event e at partition e % 128,
column e // 128.
"""

import numpy as np

E = 256
S = 16
N = 8192
NCORES = 8
ROWS = S * N // NCORES      # 16384 rows/core
G = ROWS // 128             # 128 row-groups/core
EV = N // NCORES            # 1024 events/core
GE = EV // 128              # 8 event groups
TD_HR_MAX = 5000.0
MIN_DST = 10000
UV_TILES = 16               # uv stream tiles per core
TILE_G = G // UV_TILES      # 8 groups per tile

_CACHE = {}


def _build_module():
    key = "mod"
    if key in _CACHE:
        return _CACHE[key]

    import concourse.bacc as bacc
    import concourse.tile as tile
    from concourse import mybir
    from concourse.hw_specs import get_activation_tables

    f32 = mybir.dt.float32
    bf16 = mybir.dt.bfloat16
    fp8 = mybir.dt.float8e4
    A = mybir.AluOpType
    F = mybir.ActivationFunctionType
    DR = mybir.MatmulPerfMode.DoubleRow

    class _Bacc(bacc.Bacc):
        # The stock table chooser takes the first act-table set containing
        # each function; Exp and Ln land in different sets and the ACT
        # engine thrashes ~1.3us table reloads. Put the set holding both
        # first so every activation here resolves to one table.
        def insert_act_table_loads(self):
            has_activation = any(
                isinstance(i, mybir.InstActivation)
                for b in self.main_func.blocks
                for i in b.instructions
            )
            if not has_activation:
                return
            tables = get_activation_tables(self.m.arch)
            order = [
                (name, funcs - {mybir.ActivationFunctionType.Ln}
                 if name == "natural_log" else funcs)
                for name, funcs in tables.items()
            ]
            import bass_rust as _bass_rust

            _bass_rust.insert_act_table_loads(self, order)

    nc = _Bacc(None, target_bir_lowering=False)

    # uvst[t, k, g, c2, i, m] = uv_row(t*1024 + g*128 + m)[c2*256 + i*128 + k]
    uvst_d = nc.dram_tensor("uvst", [UV_TILES, 128, TILE_G * 4 * 128], fp8,
                            kind="ExternalInput")
    # zst[k, g, ch, m] = z_row(g*128 + m)[ch*128 + k]
    zst_d = nc.dram_tensor("zst", [128, GE * 4 * 128], bf16,
                           kind="ExternalInput")
    w8_d = nc.dram_tensor("w8", [128, 4, 1], fp8, kind="ExternalInput")
    wb_d = nc.dram_tensor("wb", [128, 4], bf16, kind="ExternalInput")
    td_d = nc.dram_tensor("td", [128, G], f32, kind="ExternalInput")
    tde_d = nc.dram_tensor("tde", [128, GE], f32, kind="ExternalInput")
    sc_d = nc.dram_tensor("sc", [1, 8], f32, kind="ExternalInput")

    osurv_d = nc.dram_tensor("osurv", [128, G], f32, kind="ExternalOutput")
    oev_d = nc.dram_tensor("oev", [128, GE], f32, kind="ExternalOutput")

    with tile.TileContext(nc) as tc:
        with (
            tc.tile_pool(name="const", bufs=1) as cp,
            tc.tile_pool(name="uv", bufs=3) as up,
            tc.tile_pool(name="z", bufs=1) as zp,
            tc.tile_pool(name="post", bufs=1) as sm,
            tc.psum_pool(name="acc", bufs=1) as pp,
        ):
            # small constants on the gpsimd queue; z (1 MB) last so the
            # ACT precompute unblocks early.
            sct = cp.tile([128, 8], f32)
            nc.gpsimd.dma_start(out=sct[:], in_=sc_d[:].to_broadcast([128, 8]))
            w8t = cp.tile([128, 4, 1], fp8)
            nc.gpsimd.dma_start(out=w8t[:], in_=w8_d[:])
            wbt = cp.tile([128, 4], bf16)
            nc.gpsimd.dma_start(out=wbt[:], in_=wb_d[:])
            tdt = cp.tile([128, G], f32)
            nc.gpsimd.dma_start(out=tdt[:], in_=td_d[:])
            tdet = cp.tile([128, GE], f32)
            nc.gpsimd.dma_start(out=tdet[:], in_=tde_d[:])
            zt = zp.tile([128, GE, 4, 128], bf16)
            nc.gpsimd.dma_start(
                out=zt[:], in_=zst_d[:].rearrange("k (g c m) -> k g c m",
                                                  g=GE, c=4),
            )

            # sc columns: 0 alpha, 1 esc=-w_t/5000, 2 ivp=1/psi',
            # 3 bivp=b*ivp, 4 pclipb=75*psi'-b, 5 nclipb=-75*psi'-b
            al = sct[:, 0:1]
            esc = sct[:, 1:2]
            ivp = sct[:, 2:3]
            bivp = sct[:, 3:4]
            pclipb = sct[:, 4:5]
            nclipb = sct[:, 5:6]

            # alpha * exp(-w_t * td / 5000) precursor: et = exp(esc * td)
            et_s = sm.tile([128, G], f32)
            nc.scalar.activation(out=et_s[:], in_=tdt[:], func=F.Exp,
                                 scale=esc)
            et_e = sm.tile([128, GE], f32)
            nc.scalar.activation(out=et_e[:], in_=tdet[:], func=F.Exp,
                                 scale=esc)

            ps = pp.tile([128, G], f32)
            pse = pp.tile([128, GE], f32)

            # surv dot products: ps[m, g] = sum_d uv[row, d] * w[d]
            for t in range(UV_TILES):
                uvtile = up.tile([128, TILE_G, 2, 2, 128], fp8, tag="uvtile")
                nc.sync.dma_start(
                    out=uvtile[:],
                    in_=uvst_d[t].rearrange("k (g c i m) -> k g c i m",
                                            g=TILE_G, c=2, i=2),
                )
                for gl in range(TILE_G):
                    g = t * TILE_G + gl
                    for c2 in range(2):
                        nc.tensor.matmul(
                            out=ps[:, g:g + 1],
                            lhsT=uvtile[:, gl, c2],
                            rhs=w8t[:, 2 * c2:2 * c2 + 2, :],
                            start=(c2 == 0), stop=(c2 == 1),
                            perf_mode=DR, tile_position=(0, 0),
                        )

            # event dot products (bf16, plain): 4 chunk matmuls per group
            for g in range(GE):
                for ch in range(4):
                    nc.tensor.matmul(
                        out=pse[:, g:g + 1],
                        lhsT=zt[:, g, ch],
                        rhs=wbt[:, ch:ch + 1],
                        start=(ch == 0), stop=(ch == 3),
                        tile_position=(0, 0),
                    )

            def post(nf, g_psum, et, out_tile):
                # g2 = alpha*et + g ; y = clip(g2, nclipb, pclipb)
                # out = softplus((y + b)/psi') = Ln(1 + Exp(ivp*y + bivp))
                g2 = sm.tile([128, nf], f32, tag="g2")
                nc.vector.scalar_tensor_tensor(
                    out=g2[:], in0=et[:], scalar=al, in1=g_psum[:],
                    op0=A.mult, op1=A.add,
                )
                yc = sm.tile([128, nf], f32, tag="yc")
                nc.vector.tensor_scalar(
                    out=yc[:], in0=g2[:], scalar1=nclipb,
                    scalar2=pclipb, op0=A.max, op1=A.min,
                )
                e1 = sm.tile([128, nf], f32, tag="e1")
                nc.scalar.activation(out=e1[:], in_=yc[:], func=F.Exp,
                                     scale=ivp, bias=bivp)
                nc.scalar.activation(out=out_tile[:], in_=e1[:], func=F.Ln,
                                     bias=1.0)

            osv = sm.tile([128, G], f32)
            post(G, ps, et_s, osv)
            nc.sync.dma_start(out=osurv_d[:], in_=osv[:])

            oev = sm.tile([128, GE], f32)
            post(GE, pse, et_e, oev)
            nc.sync.dma_start(out=oev_d[:], in_=oev[:])

    nc.finalize()
    _CACHE[key] = nc
    return nc


def _stage_inputs(inputs):
    """Host-side prep: index gathers + PE-stationary per-core layouts."""
    import ml_dtypes

    bf = ml_dtypes.bfloat16
    f8 = ml_dtypes.float8_e4m3

    all_embeddings = np.asarray(inputs["all_embeddings"], dtype=np.float32)
    assoc = np.asarray(inputs["assoc"])
    src = np.asarray(inputs["src"])
    pos_dst = np.asarray(inputs["pos_dst"])
    last_update = np.asarray(inputs["last_update"], dtype=np.float32)
    cur_time = np.asarray(inputs["cur_time"], dtype=np.float32)
    u_non = np.asarray(inputs["u_non_embeddings"], dtype=np.float32)
    v_non = np.asarray(inputs["v_non_embeddings"], dtype=np.float32)
    last_time_pos = np.asarray(inputs["last_time_pos"], dtype=np.float32)
    td_surv_step = np.asarray(inputs["td_surv_step"], dtype=np.float32)
    event_inten_accu = np.asarray(inputs["event_inten_accu"], dtype=np.float32)
    W_omega = np.asarray(inputs["W_omega"], dtype=np.float32)
    b_omega = np.asarray(inputs["b_omega"], dtype=np.float32)
    psi = np.asarray(inputs["psi"], dtype=np.float32)
    alpha = np.asarray(inputs["alpha"], dtype=np.float32)
    w_t = np.asarray(inputs["w_t"], dtype=np.float32)

    idx_src = assoc[src]
    idx_dst = assoc[pos_dst]
    lu_src = last_update[idx_src]
    lu_dst = last_update[idx_dst]
    lum = np.maximum(lu_src, lu_dst)
    use_accu = (last_time_pos >= lum).astype(np.float32)
    t_uv = np.maximum(lum, last_time_pos)
    td_uv = (cur_time - t_uv).astype(np.float32)

    td_non = (td_surv_step * td_uv[None, :]).astype(np.float32)  # (S, N)
    accu_g = event_inten_accu[src, pos_dst - MIN_DST].astype(np.float32)

    uv8 = np.empty((S * N, 2 * E), dtype=f8)
    uv8[:, :E] = u_non
    uv8[:, E:] = v_non

    zb = np.empty((N, 2 * E), dtype=bf)
    zb[:, :E] = all_embeddings[idx_src]
    zb[:, E:] = all_embeddings[idx_dst]

    w = W_omega.reshape(2 * E)
    # w8[k, 2*c2 + i] = w[c2*256 + i*128 + k]
    w8 = np.ascontiguousarray(
        w.reshape(2, 2, 128).transpose(2, 0, 1).reshape(128, 4, 1)
    ).astype(f8)
    # wb[k, ch] = w[ch*128 + k]
    wb = np.ascontiguousarray(w.reshape(4, 128).T).astype(bf)

    psi_p = float(psi[0]) + 1e-7
    b = float(b_omega[0])
    ivp = 1.0 / psi_p
    sc = np.array([[float(alpha[0]), -float(w_t[0]) / TD_HR_MAX, ivp,
                    b * ivp, 75.0 * psi_p - b, -75.0 * psi_p - b, 0.0, 0.0]],
                  dtype=np.float32)

    in_maps = []
    for c in range(NCORES):
        arr = uv8[c * ROWS:(c + 1) * ROWS]            # [16384, 512]
        # [t, k, g, c2, i, m] <- arr[t*1024 + g*128 + m, c2*256 + i*128 + k]
        uvst = np.ascontiguousarray(
            arr.reshape(UV_TILES, TILE_G, 128, 2, 2, 128)
               .transpose(0, 5, 1, 3, 4, 2)
               .reshape(UV_TILES, 128, TILE_G * 4 * 128)
        )
        ze = zb[c * EV:(c + 1) * EV]                  # [1024, 512]
        # [k, g, ch, m] <- ze[g*128 + m, ch*128 + k]
        zst = np.ascontiguousarray(
            ze.reshape(GE, 128, 4, 128).transpose(3, 0, 2, 1)
              .reshape(128, GE * 4 * 128)
        )
        td_core = td_non[2 * c:2 * c + 2, :].reshape(G, 128)   # r = g*128+m
        tde_core = td_uv[c * EV:(c + 1) * EV].reshape(GE, 128)
        in_maps.append(
            dict(uvst=uvst, zst=zst, w8=w8, wb=wb,
                 td=np.ascontiguousarray(td_core.T),
                 tde=np.ascontiguousarray(tde_core.T), sc=sc)
        )
    return in_maps, td_uv, use_accu, accu_g, float(psi[0])


def _combine(results, td_uv, use_accu, accu_g, psi_val):
    sp_sum = np.zeros(N, dtype=np.float64)
    lam_ev = np.empty(N, dtype=np.float64)
    for c, r in enumerate(results):
        o = np.asarray(r["osurv"], dtype=np.float64)   # [128 m, 128 g]
        sp_sum += o.T.reshape(2, N).sum(axis=0)
        lam_ev[c * EV:(c + 1) * EV] = np.asarray(
            r["oev"], dtype=np.float64
        ).T.reshape(EV)

    mean_lambda_surv = psi_val * (sp_sum / S)
    integral = mean_lambda_surv * td_uv.astype(np.float64) + use_accu.astype(
        np.float64
    ) * accu_g.astype(np.float64)
    loss_surv = integral.sum() / N

    lam_uv = psi_val * lam_ev
    loss_lambda = -np.log(lam_uv + 1e-7).sum() / N
    return np.float32(loss_lambda), np.float32(loss_surv)


def _run(in_maps, trace=False):
    from concourse.bass_utils import run_bass_kernel_spmd

    nc = _build_module()
    res = run_bass_kernel_spmd(
        nc, in_maps, core_ids=list(range(NCORES)), trace=trace
    )
    return res


def kernel(**inputs):
    in_maps, td_uv, use_accu, accu_g, psi_val = _stage_inputs(inputs)
    res = _run(in_maps)
    return _combine(res.results, td_uv, use_accu, accu_g, psi_val)


def kernel_traced(**inputs):
    """Like kernel() but also returns the HW exec time in ns (test harness)."""
    in_maps, td_uv, use_accu, accu_g, psi_val = _stage_inputs(inputs)
    res = _run(in_maps, trace=True)
    out = _combine(res.results, td_uv, use_accu, accu_g, psi_val)
    return out, res.exec_time_ns
